# revision 16
# baseline (speedup 1.0000x reference)
"""RoPE + ALiBi attention (B=2, T=2048, H=1024, 16 heads) on 8 trn2 cores.

Strategy
--------
ALiBi bias s_h*(k - q) is, for every query, maximal at the last key
(k = T-1).  Keys with s_h*(T-1-k) > MARGIN contribute negligibly and
are dropped: per-head key windows of 1..16 tiles of 128 keys.  Softmax
runs without a max pass: exp(qk/8) directly, with the ALiBi factor
e^{s(k-(T-1))} folded into host-prescaled V rows; the denominator
comes from a 65th V column holding e^{s(k-(T-1))}.

All PE work in bf16 (fp32 matmuls trigger PE power-throttle and run
at <= 1.2 GHz; bf16 streams 1 row/cycle and can ramp to 2.4 GHz).

SPMD: one program, 8 cores.  Core c handles batch c//4, query-quarter
c%4 (512 queries) of ALL 16 heads.

Schedule: software-pipelined.  Per head: S^T group (PE) -> exp (ACT)
-> PV delayed one group so PE never round-trips on ACT.  Prep for the
next head (kT/qT transposes) and finalize of the previous head are
interleaved as PE filler between groups.  RoPE k on Pool, RoPE q and
copies on DVE.  DMAs are split per head so early heads start
immediately.
"""

import ml_dtypes
import numpy as np

import concourse.bass as bass
import concourse.bacc as bacc
import concourse.tile as tile
import concourse.mybir as mybir
from concourse.bass_utils import run_bass_kernel_spmd
from concourse.masks import make_identity
from concourse._compat import get_trn_type

F32 = mybir.dt.float32
BF16 = mybir.dt.bfloat16
MMDT = mybir.dt.bfloat16

B, T, H = 2, 2048, 1024
NH, HD = 16, 64
NCORES = 8
NQT_SLOT = 4              # 512 queries per slot = 4 tiles of 128
MARGIN = 12.0             # ALiBi window cut (windowing err ~1.4e-5)
EXP_GROUP = 2             # k-tiles per exp() batch

SLOPES = np.array([2.0 ** (-8.0 * i / NH) for i in range(1, NH + 1)], np.float64)
# process small and large windows interleaved; output slots are in
# PROCESSING order (host descatters)
HEAD_ORDER = [0, 15, 1, 14, 2, 13, 3, 12, 4, 11, 5, 10, 6, 9, 7, 8]
WT = [min(T // 128, int(np.ceil((MARGIN / s + 1) / 128))) for s in SLOPES]
KOFF = np.concatenate([[0], np.cumsum(WT)]).astype(int)
NKT = int(KOFF[-1])       # total k-tiles per core


def _rope_tables():
    inv = 1.0 / (10000.0 ** (np.arange(0, HD, 2, dtype=np.float64) / HD))
    fr = np.outer(np.arange(T, dtype=np.float64), inv)        # [T, 32]
    emb = np.concatenate([fr, fr], axis=-1)                   # [T, 64]
    cos = np.cos(emb).astype(np.float32)
    sinr = np.sin(emb).astype(np.float32)
    sinr[:, : HD // 2] *= -1.0          # fold rotate-half sign into the table
    return cos, sinr


def _build_program():
    nc = bacc.Bacc(get_trn_type() or "TRN2", target_bir_lowering=False, debug=False)

    qg_d = nc.dram_tensor("q_g", [128, NH * NQT_SLOT, HD], BF16, kind="ExternalInput")
    kg_d = nc.dram_tensor("k_g", [128, NKT, HD], BF16, kind="ExternalInput")
    vg_d = nc.dram_tensor("v_g", [128, NKT, HD + 1], MMDT, kind="ExternalInput")
    cq_d = nc.dram_tensor("cos_q", [128, NQT_SLOT, HD], BF16, kind="ExternalInput")
    sq_d = nc.dram_tensor("sin_q", [128, NQT_SLOT, HD], BF16, kind="ExternalInput")
    ck_d = nc.dram_tensor("cos_k", [128, T // 128, HD], BF16, kind="ExternalInput")
    sk_d = nc.dram_tensor("sin_k", [128, T // 128, HD], BF16, kind="ExternalInput")
    og_d = nc.dram_tensor("out_g", [128, NH * NQT_SLOT, HD], F32, kind="ExternalOutput")

    hd2 = HD // 2

    with tile.TileContext(nc) as tc:
        with (
            tc.tile_pool(name="singles", bufs=1) as singles,
            tc.tile_pool(name="rope", bufs=2) as rope_pool,
            tc.tile_pool(name="krp", bufs=3) as kr_pool,
            tc.tile_pool(name="qkt", bufs=3) as qkt_pool,
            tc.tile_pool(name="qtp", bufs=3) as qt_pool,
            tc.tile_pool(name="pt", bufs=3) as pt_pool,
            tc.tile_pool(name="fin", bufs=2) as fin_pool,
            tc.tile_pool(name="ps_t", bufs=2, space="PSUM") as ps_t,
            tc.tile_pool(name="ps_s", bufs=2, space="PSUM") as ps_s,
            tc.tile_pool(name="ps_o", bufs=2, space="PSUM") as ps_o,
        ):
            ident = singles.tile([128, 128], BF16)
            make_identity(nc, ident[:])

            q_sbs = [singles.tile([128, 16, HD], BF16, tag=f"qsb{g}", name=f"qsb{g}")
                     for g in range(4)]
            k_sb = singles.tile([128, NKT, HD], BF16, tag="ksb", name="ksb")
            v_sb = singles.tile([128, NKT, HD + 1], MMDT, tag="vsb", name="vsb")
            cq = singles.tile([128, NQT_SLOT, HD], BF16)
            sq = singles.tile([128, NQT_SLOT, HD], BF16)
            ck = singles.tile([128, T // 128, HD], BF16)
            sk = singles.tile([128, T // 128, HD], BF16)

            # ---- DMAs: tables+q first, then per-head k/v chunks ----
            nc.sync.dma_start(out=cq, in_=cq_d[:])
            nc.sync.dma_start(out=sq, in_=sq_d[:])
            nc.sync.dma_start(out=ck, in_=ck_d[:])
            nc.sync.dma_start(out=sk, in_=sk_d[:])
            for g in range(4):
                nc.sync.dma_start(out=q_sbs[g], in_=qg_d[:, 16 * g:16 * (g + 1), :])
            for h in HEAD_ORDER:
                w, ko = WT[h], KOFF[h]
                nc.sync.dma_start(out=k_sb[:, ko:ko + w, :],
                                  in_=kg_d[:, ko:ko + w, :])
                nc.sync.dma_start(out=v_sb[:, ko:ko + w, :],
                                  in_=vg_d[:, ko:ko + w, :])

            def rope4(eng, dst, src, cos_, sinr_, shape, pool=rope_pool):
                # dst = src*cos + swap_halves(src)*sin_rot   (4 ops)
                def lo(ap):
                    return ap[(slice(None),) * (len(shape) - 1) + (slice(0, hd2),)]

                def hi(ap):
                    return ap[(slice(None),) * (len(shape) - 1) + (slice(hd2, HD),)]

                t = pool.tile(shape, BF16, tag="ropet", name="ropet")
                u = pool.tile(shape, BF16, tag="ropeu", name="ropeu")
                eng.tensor_mul(t, src, cos_)
                eng.tensor_mul(lo(u), hi(src), lo(sinr_))
                eng.tensor_mul(hi(u), lo(src), hi(sinr_))
                eng.tensor_add(dst, t, u)

            # ---- bulk RoPE q: 4 groups of 4 slots; cos broadcast over slots
            qr_gs = [singles.tile([128, 16, HD], BF16, tag=f"qr{g}", name=f"qr{g}")
                     for g in range(4)]
            cqa, sqa = cq[:], sq[:]
            cq4 = bass.AP(tensor=cqa.tensor, offset=cqa.offset,
                          ap=[list(cqa.ap[0]), [0, 4]] + [list(a) for a in cqa.ap[1:]])
            sq4 = bass.AP(tensor=sqa.tensor, offset=sqa.offset,
                          ap=[list(sqa.ap[0]), [0, 4]] + [list(a) for a in sqa.ap[1:]])
            for g in range(4):
                src = q_sbs[g][:, :, :].rearrange("p (s n) d -> p s n d", s=4)
                dst = qr_gs[g][:, :, :].rearrange("p (s n) d -> p s n d", s=4)
                eng = nc.vector if g < 2 else nc.gpsimd
                rope4(eng, dst, src, cq4, sq4, [128, 4, NQT_SLOT, HD])

            # ---- per-head helpers ----
            kr_map = {}

            def rope_k(h, eng):
                w, ko = WT[h], KOFF[h]
                a0 = T // 128 - w
                kr = kr_pool.tile([128, 16, HD], BF16, tag="kr", name="kr")
                rope4(eng, kr[:, 0:w, :], k_sb[:, ko:ko + w, :],
                      ck[:, a0:a0 + w, :], sk[:, a0:a0 + w, :], [128, w, HD],
                      pool=kr_pool)
                kr_map[h] = kr

            qT_map = {}
            kT_map = {}

            def prep_fillers(h):
                """PE filler chunks building qT and kT for head h."""
                fillers = []

                def qT_prep():
                    qt_ps = ps_t.tile([64, 512], BF16, tag="tp", name="qt_ps")
                    for n in range(NQT_SLOT):
                        nc.tensor.transpose(
                            qt_ps[:, n * 128:(n + 1) * 128],
                            qr_gs[h // 4][:, NQT_SLOT * (h % 4) + n, :], ident[:])
                    qT = qt_pool.tile([64, 512], MMDT, tag="qT", name="qT")
                    nc.vector.tensor_copy(qT, qt_ps)
                    qT_map[h] = qT

                fillers.append(qT_prep)
                w = WT[h]
                kT = qkt_pool.tile([64, 16 * 128], MMDT, tag="kT", name="kT")
                kT_map[h] = kT

                def kT_chunk(g0):
                    gn = min(4, w - g0)
                    kr = kr_map[h]
                    kt_ps = ps_t.tile([64, 512], BF16, tag="tp", name="kt_ps")
                    for j in range(gn):
                        nc.tensor.transpose(kt_ps[:, j * 128:(j + 1) * 128],
                                            kr[:, g0 + j, :], ident[:])
                    nc.vector.tensor_copy(kT[:, g0 * 128:(g0 + gn) * 128],
                                          kt_ps[:, 0:gn * 128])

                for g0 in range(0, w, 4):
                    fillers.append(lambda g0=g0: kT_chunk(g0))
                return fillers

            out_f_box = [None]

            def finalize(hi, h):
                """oT copy (DVE) -> 4 out-transposes (PE) -> recip/mul -> DMA."""
                o_ps = ops_map.pop(h)
                oT = fin_pool.tile([HD + 1, 512], BF16, tag="oT", name="oT")
                if hi % 2 == 0:
                    nc.scalar.copy(oT, o_ps)
                else:
                    nc.vector.tensor_copy(oT, o_ps)
                of_ps = ps_t.tile([128, NQT_SLOT, HD + 2], BF16, tag="tp",
                                  name="of_ps")
                for n in range(NQT_SLOT):
                    nc.tensor.transpose(of_ps[:, n, 0:HD + 1],
                                        oT[:, n * 128:(n + 1) * 128],
                                        ident[0:HD + 1, 0:HD + 1])
                rec = fin_pool.tile([128, NQT_SLOT, 1], F32, tag="rec", name="rec")
                nc.vector.reciprocal(rec, of_ps[:, :, HD:HD + 1])
                reca = rec[:, :, 0:1]
                rec_b = bass.AP(tensor=reca.tensor, offset=reca.offset,
                                ap=[list(reca.ap[0]), list(reca.ap[1]), [0, HD]])
                if hi % 4 == 0:
                    out_f_box[0] = fin_pool.tile([128, 4 * NQT_SLOT, HD], F32,
                                                 tag="of", name="of")
                out_f = out_f_box[0]
                hh = hi % 4
                nc.vector.tensor_mul(out_f[:, NQT_SLOT * hh:NQT_SLOT * (hh + 1), :],
                                     of_ps[:, :, 0:HD], rec_b)
                if hi % 4 == 3:
                    nc.sync.dma_start(
                        out=og_d[:, NQT_SLOT * (hi - 3):NQT_SLOT * (hi + 1), :],
                        in_=out_f)

            ops_map = {}

            # ---- RoPE k for first two heads; prep first head up front ----
            rope_k(HEAD_ORDER[0], nc.gpsimd)
            rope_k(HEAD_ORDER[1], nc.gpsimd)
            for f in prep_fillers(HEAD_ORDER[0]):
                f()

            # ---- main loop over heads ----
            for hi, h in enumerate(HEAD_ORDER):
                w, ko = WT[h], KOFF[h]
                qT = qT_map.pop(h)
                kT = kT_map[h]

                if hi + 2 < NH:
                    rope_k(HEAD_ORDER[hi + 2], nc.gpsimd)

                fillers = []
                if hi + 1 < NH:
                    fillers.extend(prep_fillers(HEAD_ORDER[hi + 1]))
                if hi > 0:
                    fillers.append(lambda: finalize(hi - 1, HEAD_ORDER[hi - 1]))

                def kT_slice(j):
                    return kT[:, j * 128:(j + 1) * 128]

                o_ps = ps_o.tile([HD + 1, 512], F32, tag="ops", name="o_ps")
                ops_map[h] = o_ps

                ngroups = (w + EXP_GROUP - 1) // EXP_GROUP
                pend = None
                for gi in range(ngroups):
                    g0 = gi * EXP_GROUP
                    gn = min(EXP_GROUP, w - g0)
                    st_ps = ps_s.tile([128, EXP_GROUP * 512], F32, tag="st",
                                      name="st_ps")
                    for j in range(gn):
                        nc.tensor.matmul(
                            st_ps[:, j * 512:(j + 1) * 512],
                            lhsT=kT_slice(g0 + j), rhs=qT,
                            start=True, stop=True,
                        )
                    pT = pt_pool.tile([128, EXP_GROUP * 512], MMDT, tag="pT",
                                      name="pT")
                    nc.scalar.activation(
                        out=pT[:, 0:gn * 512], in_=st_ps[:, 0:gn * 512],
                        func=mybir.ActivationFunctionType.Exp,
                        bias=0.0, scale=0.125,
                    )
                    if fillers:
                        fillers.pop(0)()
                    if pend is not None:
                        pg0, pgn, ppT = pend
                        for j in range(pgn):
                            nc.tensor.matmul(
                                o_ps,
                                lhsT=v_sb[:, ko + pg0 + j, :],
                                rhs=ppT[:, j * 512:(j + 1) * 512],
                                start=(pg0 + j == 0), stop=False,
                                skip_group_check=True,
                            )
                    pend = (g0, gn, pT)
                pg0, pgn, ppT = pend
                for j in range(pgn):
                    nc.tensor.matmul(
                        o_ps,
                        lhsT=v_sb[:, ko + pg0 + j, :],
                        rhs=ppT[:, j * 512:(j + 1) * 512],
                        start=(pg0 + j == 0), stop=(pg0 + j == w - 1),
                        skip_group_check=True,
                    )
                for f in fillers:
                    f()

            finalize(NH - 1, HEAD_ORDER[NH - 1])

    nc.compile()
    return nc


_PROGRAM = None
TRACE = False
LAST_RESULT = None


def kernel(q, k, v, num_heads=16):
    global _PROGRAM
    q = np.ascontiguousarray(np.asarray(q, dtype=np.float32))
    k = np.ascontiguousarray(np.asarray(k, dtype=np.float32))
    v = np.ascontiguousarray(np.asarray(v, dtype=np.float32))

    BF = ml_dtypes.bfloat16
    cos, sinr = _rope_tables()
    ck = np.ascontiguousarray(
        cos.reshape(T // 128, 128, HD).transpose(1, 0, 2)).astype(BF)
    sk = np.ascontiguousarray(
        sinr.reshape(T // 128, 128, HD).transpose(1, 0, 2)).astype(BF)

    in_maps = []
    for c in range(NCORES):
        b, qq = c // 4, c % 4
        qg = np.empty((128, NH * NQT_SLOT, HD), np.float32)
        kg = np.empty((128, NKT, HD), np.float32)
        vg = np.empty((128, NKT, HD + 1), np.float32)
        for h in range(NH):
            w, ko = WT[h], KOFF[h]
            a0 = T // 128 - w
            qs = q[b, qq * 512:(qq + 1) * 512, h * HD:(h + 1) * HD]
            qg[:, NQT_SLOT * h:NQT_SLOT * (h + 1), :] = (
                qs.reshape(NQT_SLOT, 128, HD).transpose(1, 0, 2))
            ks = k[b, a0 * 128:T, h * HD:(h + 1) * HD]
            kg[:, ko:ko + w, :] = ks.reshape(w, 128, HD).transpose(1, 0, 2)
            vs = v[b, a0 * 128:T, h * HD:(h + 1) * HD]
            eb = np.exp(SLOPES[h] * (np.arange(a0 * 128, T, dtype=np.float64)
                                     - (T - 1.0))).astype(np.float32)
            vsc = (vs * eb[:, None]).reshape(w, 128, HD).transpose(1, 0, 2)
            vg[:, ko:ko + w, 0:HD] = vsc
            vg[:, ko:ko + w, HD] = eb.reshape(w, 128).T
        cqg = np.ascontiguousarray(
            cos[qq * 512:(qq + 1) * 512].reshape(NQT_SLOT, 128, HD)
            .transpose(1, 0, 2)).astype(BF)
        sqg = np.ascontiguousarray(
            sinr[qq * 512:(qq + 1) * 512].reshape(NQT_SLOT, 128, HD)
            .transpose(1, 0, 2)).astype(BF)
        in_maps.append({
            "q_g": qg.astype(BF), "k_g": kg.astype(BF), "v_g": vg.astype(BF),
            "cos_q": cqg, "sin_q": sqg, "cos_k": ck, "sin_k": sk,
        })

    if _PROGRAM is None:
        _PROGRAM = _build_program()

    global LAST_RESULT
    res = run_bass_kernel_spmd(_PROGRAM, in_maps, core_ids=list(range(NCORES)),
                               trace=TRACE)
    LAST_RESULT = res

    out = np.empty((B, T, H), np.float32)
    for c in range(NCORES):
        b, qq = c // 4, c % 4
        og = res.results[c]["out_g"]
        for hi, h in enumerate(HEAD_ORDER):
            sl = og[:, NQT_SLOT * hi:NQT_SLOT * (hi + 1), :]   # [128, 4, 64]
            out[b, qq * 512:(qq + 1) * 512, h * HD:(h + 1) * HD] = (
                sl.transpose(1, 0, 2).reshape(512, HD))
    return out


# revision 21
# speedup vs baseline: 1.0246x; 1.0246x over previous
"""RoPE + ALiBi attention (B=2, T=2048, H=1024, 16 heads) on 8 trn2 cores.

Strategy
--------
ALiBi bias s_h*(k - q) is, for every query, maximal at the last key
(k = T-1).  Keys with s_h*(T-1-k) > MARGIN contribute negligibly and
are dropped: per-head key windows of 1..16 tiles of 128 keys.  Softmax
runs without a max pass: exp(qk/8) directly, with the ALiBi factor
e^{s(k-(T-1))} folded into host-prescaled V rows; the denominator
comes from a 65th V column holding e^{s(k-(T-1))}.

All PE work in bf16 (fp32 matmuls trigger PE power-throttle and run
at <= 1.2 GHz; bf16 streams 1 row/cycle and can ramp to 2.4 GHz).

SPMD: one program, 8 cores.  Core c handles batch c//4, query-quarter
c%4 (512 queries) of ALL 16 heads.

Schedule: software-pipelined.  Per head: S^T group (PE) -> exp (ACT)
-> PV delayed one group so PE never round-trips on ACT.  Prep for the
next head (kT/qT transposes) and finalize of the previous head are
interleaved as PE filler between groups.  RoPE k on Pool, RoPE q and
copies on DVE.  DMAs are split per head so early heads start
immediately.
"""

import ml_dtypes
import numpy as np

import concourse.bass as bass
import concourse.bacc as bacc
import concourse.tile as tile
import concourse.mybir as mybir
from concourse.bass_utils import run_bass_kernel_spmd
from concourse.masks import make_identity
from concourse._compat import get_trn_type

F32 = mybir.dt.float32
BF16 = mybir.dt.bfloat16
MMDT = mybir.dt.bfloat16

B, T, H = 2, 2048, 1024
NH, HD = 16, 64
NCORES = 8
NQT_SLOT = 4              # 512 queries per slot = 4 tiles of 128
MARGIN = 12.0             # ALiBi window cut (windowing err ~1.4e-5)
EXP_GROUP = 2             # k-tiles per exp() batch

SLOPES = np.array([2.0 ** (-8.0 * i / NH) for i in range(1, NH + 1)], np.float64)
# process small and large windows interleaved; output slots are in
# PROCESSING order (host descatters)
HEAD_ORDER = [0, 15, 1, 14, 2, 13, 3, 12, 4, 11, 5, 10, 6, 9, 7, 8]
WT = [min(T // 128, int(np.ceil((MARGIN / s + 1) / 128))) for s in SLOPES]
KOFF = np.concatenate([[0], np.cumsum(WT)]).astype(int)
NKT = int(KOFF[-1])       # total k-tiles per core


def _rope_tables():
    inv = 1.0 / (10000.0 ** (np.arange(0, HD, 2, dtype=np.float64) / HD))
    fr = np.outer(np.arange(T, dtype=np.float64), inv)        # [T, 32]
    emb = np.concatenate([fr, fr], axis=-1)                   # [T, 64]
    cos = np.cos(emb).astype(np.float32)
    sinr = np.sin(emb).astype(np.float32)
    sinr[:, : HD // 2] *= -1.0          # fold rotate-half sign into the table
    return cos, sinr


def _build_program():
    nc = bacc.Bacc(get_trn_type() or "TRN2", target_bir_lowering=False, debug=False)

    qg_d = nc.dram_tensor("q_g", [128, NH * NQT_SLOT, HD], BF16, kind="ExternalInput")
    kg_d = nc.dram_tensor("k_g", [128, NKT, HD], BF16, kind="ExternalInput")
    vg_d = nc.dram_tensor("v_g", [128, NKT, HD + 1], MMDT, kind="ExternalInput")
    cq_d = nc.dram_tensor("cos_q", [128, NQT_SLOT, HD], BF16, kind="ExternalInput")
    sq_d = nc.dram_tensor("sin_q", [128, NQT_SLOT, HD], BF16, kind="ExternalInput")
    ck_d = nc.dram_tensor("cos_k", [128, T // 128, HD], BF16, kind="ExternalInput")
    sk_d = nc.dram_tensor("sin_k", [128, T // 128, HD], BF16, kind="ExternalInput")
    og_d = nc.dram_tensor("out_g", [128, NH * NQT_SLOT, HD], F32, kind="ExternalOutput")

    hd2 = HD // 2

    with tile.TileContext(nc) as tc:
        with (
            tc.tile_pool(name="singles", bufs=1) as singles,
            tc.tile_pool(name="rope", bufs=2) as rope_pool,
            tc.tile_pool(name="krp", bufs=3) as kr_pool,
            tc.tile_pool(name="qkt", bufs=3) as qkt_pool,
            tc.tile_pool(name="qtp", bufs=3) as qt_pool,
            tc.tile_pool(name="pt", bufs=3) as pt_pool,
            tc.tile_pool(name="fin", bufs=2) as fin_pool,
            tc.tile_pool(name="ps_t", bufs=2, space="PSUM") as ps_t,
            tc.tile_pool(name="ps_s", bufs=2, space="PSUM") as ps_s,
            tc.tile_pool(name="ps_o", bufs=2, space="PSUM") as ps_o,
        ):
            ident = singles.tile([128, 128], BF16)
            make_identity(nc, ident[:])

            q_sbs = [singles.tile([128, 16, HD], BF16, tag=f"qsb{g}", name=f"qsb{g}")
                     for g in range(4)]
            k_sb = singles.tile([128, NKT, HD], BF16, tag="ksb", name="ksb")
            v_sb = singles.tile([128, NKT, HD + 1], MMDT, tag="vsb", name="vsb")
            cq = singles.tile([128, NQT_SLOT, HD], BF16)
            sq = singles.tile([128, NQT_SLOT, HD], BF16)
            ck = singles.tile([128, T // 128, HD], BF16)
            sk = singles.tile([128, T // 128, HD], BF16)

            # ---- DMAs ordered by first use on the startup critical path ----
            def dma_q(g):
                nc.sync.dma_start(out=q_sbs[g], in_=qg_d[:, 16 * g:16 * (g + 1), :])

            def dma_kv(h):
                w, ko = WT[h], KOFF[h]
                nc.sync.dma_start(out=k_sb[:, ko:ko + w, :],
                                  in_=kg_d[:, ko:ko + w, :])
                nc.sync.dma_start(out=v_sb[:, ko:ko + w, :],
                                  in_=vg_d[:, ko:ko + w, :])

            nc.sync.dma_start(out=cq, in_=cq_d[:])
            nc.sync.dma_start(out=sq, in_=sq_d[:])
            dma_q(0)
            nc.sync.dma_start(out=ck, in_=ck_d[:])
            nc.sync.dma_start(out=sk, in_=sk_d[:])
            dma_kv(HEAD_ORDER[0])
            dma_q(3)
            dma_kv(HEAD_ORDER[1])
            dma_q(1)
            dma_q(2)
            for h in HEAD_ORDER[2:]:
                dma_kv(h)

            def rope4(eng, dst, src, cos_, sinr_, shape, pool=rope_pool):
                # dst = src*cos + swap_halves(src)*sin_rot   (4 ops)
                def lo(ap):
                    return ap[(slice(None),) * (len(shape) - 1) + (slice(0, hd2),)]

                def hi(ap):
                    return ap[(slice(None),) * (len(shape) - 1) + (slice(hd2, HD),)]

                t = pool.tile(shape, BF16, tag="ropet", name="ropet")
                u = pool.tile(shape, BF16, tag="ropeu", name="ropeu")
                eng.tensor_mul(t, src, cos_)
                eng.tensor_mul(lo(u), hi(src), lo(sinr_))
                eng.tensor_mul(hi(u), lo(src), hi(sinr_))
                eng.tensor_add(dst, t, u)

            # ---- bulk RoPE q: 4 groups of 4 slots; cos broadcast over slots
            qr_gs = [singles.tile([128, 16, HD], BF16, tag=f"qr{g}", name=f"qr{g}")
                     for g in range(4)]
            cqa, sqa = cq[:], sq[:]
            cq4 = bass.AP(tensor=cqa.tensor, offset=cqa.offset,
                          ap=[list(cqa.ap[0]), [0, 4]] + [list(a) for a in cqa.ap[1:]])
            sq4 = bass.AP(tensor=sqa.tensor, offset=sqa.offset,
                          ap=[list(sqa.ap[0]), [0, 4]] + [list(a) for a in sqa.ap[1:]])
            def rope_q(g, eng):
                src = q_sbs[g][:, :, :].rearrange("p (s n) d -> p s n d", s=4)
                dst = qr_gs[g][:, :, :].rearrange("p (s n) d -> p s n d", s=4)
                rope4(eng, dst, src, cq4, sq4, [128, 4, NQT_SLOT, HD])

            # ---- per-head helpers ----
            kr_map = {}

            def rope_k(h, eng):
                w, ko = WT[h], KOFF[h]
                a0 = T // 128 - w
                kr = kr_pool.tile([128, 16, HD], BF16, tag="kr", name="kr")
                rope4(eng, kr[:, 0:w, :], k_sb[:, ko:ko + w, :],
                      ck[:, a0:a0 + w, :], sk[:, a0:a0 + w, :], [128, w, HD],
                      pool=kr_pool)
                kr_map[h] = kr

            qT_map = {}
            kT_map = {}

            def prep_fillers(h):
                """PE filler chunks building qT and kT for head h."""
                fillers = []

                def qT_prep():
                    qt_ps = ps_t.tile([64, 512], BF16, tag="tp", name="qt_ps")
                    for n in range(NQT_SLOT):
                        nc.tensor.transpose(
                            qt_ps[:, n * 128:(n + 1) * 128],
                            qr_gs[h // 4][:, NQT_SLOT * (h % 4) + n, :], ident[:])
                    qT = qt_pool.tile([64, 512], MMDT, tag="qT", name="qT")
                    nc.vector.tensor_copy(qT, qt_ps)
                    qT_map[h] = qT

                fillers.append(qT_prep)
                w = WT[h]
                kT = qkt_pool.tile([64, 16 * 128], MMDT, tag="kT", name="kT")
                kT_map[h] = kT

                def kT_chunk(g0):
                    gn = min(4, w - g0)
                    kr = kr_map[h]
                    kt_ps = ps_t.tile([64, 512], BF16, tag="tp", name="kt_ps")
                    for j in range(gn):
                        nc.tensor.transpose(kt_ps[:, j * 128:(j + 1) * 128],
                                            kr[:, g0 + j, :], ident[:])
                    nc.vector.tensor_copy(kT[:, g0 * 128:(g0 + gn) * 128],
                                          kt_ps[:, 0:gn * 128])

                for g0 in range(0, w, 4):
                    fillers.append(lambda g0=g0: kT_chunk(g0))
                return fillers

            def finalize(hi, h):
                """oT copy (DVE) -> 4 out-transposes (PE) -> recip/mul -> DMA."""
                o_ps = ops_map.pop(h)
                oT = fin_pool.tile([HD + 1, 512], BF16, tag="oT", name="oT")
                nc.vector.tensor_copy(oT, o_ps)
                of_ps = ps_t.tile([128, NQT_SLOT, HD + 2], BF16, tag="tp",
                                  name="of_ps")
                for n in range(NQT_SLOT):
                    nc.tensor.transpose(of_ps[:, n, 0:HD + 1],
                                        oT[:, n * 128:(n + 1) * 128],
                                        ident[0:HD + 1, 0:HD + 1])
                rec = fin_pool.tile([128, NQT_SLOT, 1], F32, tag="rec", name="rec")
                nc.vector.reciprocal(rec, of_ps[:, :, HD:HD + 1])
                reca = rec[:, :, 0:1]
                rec_b = bass.AP(tensor=reca.tensor, offset=reca.offset,
                                ap=[list(reca.ap[0]), list(reca.ap[1]), [0, HD]])
                out_f = fin_pool.tile([128, NQT_SLOT, HD], F32, tag="of",
                                      name="of")
                nc.vector.tensor_mul(out_f, of_ps[:, :, 0:HD], rec_b)
                nc.sync.dma_start(
                    out=og_d[:, NQT_SLOT * hi:NQT_SLOT * (hi + 1), :],
                    in_=out_f)

            ops_map = {}

            # ---- startup: interleave RoPE q groups and the first two heads'
            # RoPE k across DVE/Pool so head 0 and head 15 unblock fast ----
            rope_q(0, nc.vector)       # head 0 lives in q group 0
            rope_q(3, nc.gpsimd)       # head 15 lives in q group 3
            rope_k(HEAD_ORDER[0], nc.vector)   # tiny (w=1)
            rope_k(HEAD_ORDER[1], nc.gpsimd)
            for f in prep_fillers(HEAD_ORDER[0]):
                f()
            rope_q(1, nc.vector)
            rope_q(2, nc.gpsimd)

            # ---- main loop over heads ----
            for hi, h in enumerate(HEAD_ORDER):
                w, ko = WT[h], KOFF[h]
                qT = qT_map.pop(h)
                kT = kT_map[h]

                if hi + 2 < NH:
                    rope_k(HEAD_ORDER[hi + 2], nc.gpsimd)

                fillers = []
                if hi + 1 < NH:
                    fillers.extend(prep_fillers(HEAD_ORDER[hi + 1]))
                if hi > 0:
                    fillers.append(lambda: finalize(hi - 1, HEAD_ORDER[hi - 1]))

                def kT_slice(j):
                    return kT[:, j * 128:(j + 1) * 128]

                o_ps = ps_o.tile([HD + 1, 512], F32, tag="ops", name="o_ps")
                ops_map[h] = o_ps

                ngroups = (w + EXP_GROUP - 1) // EXP_GROUP
                pend = None
                for gi in range(ngroups):
                    g0 = gi * EXP_GROUP
                    gn = min(EXP_GROUP, w - g0)
                    st_ps = ps_s.tile([128, EXP_GROUP * 512], F32, tag="st",
                                      name="st_ps")
                    for j in range(gn):
                        nc.tensor.matmul(
                            st_ps[:, j * 512:(j + 1) * 512],
                            lhsT=kT_slice(g0 + j), rhs=qT,
                            start=True, stop=True,
                        )
                    pT = pt_pool.tile([128, EXP_GROUP * 512], MMDT, tag="pT",
                                      name="pT")
                    nc.scalar.activation(
                        out=pT[:, 0:gn * 512], in_=st_ps[:, 0:gn * 512],
                        func=mybir.ActivationFunctionType.Exp,
                        bias=0.0, scale=0.125,
                    )
                    if fillers:
                        fillers.pop(0)()
                    if pend is not None:
                        pg0, pgn, ppT = pend
                        for j in range(pgn):
                            nc.tensor.matmul(
                                o_ps,
                                lhsT=v_sb[:, ko + pg0 + j, :],
                                rhs=ppT[:, j * 512:(j + 1) * 512],
                                start=(pg0 + j == 0), stop=False,
                                skip_group_check=True,
                            )
                    pend = (g0, gn, pT)
                pg0, pgn, ppT = pend
                for j in range(pgn):
                    nc.tensor.matmul(
                        o_ps,
                        lhsT=v_sb[:, ko + pg0 + j, :],
                        rhs=ppT[:, j * 512:(j + 1) * 512],
                        start=(pg0 + j == 0), stop=(pg0 + j == w - 1),
                        skip_group_check=True,
                    )
                for f in fillers:
                    f()

            finalize(NH - 1, HEAD_ORDER[NH - 1])

    nc.compile()
    return nc


_PROGRAM = None
TRACE = False
LAST_RESULT = None


def kernel(q, k, v, num_heads=16):
    global _PROGRAM
    q = np.ascontiguousarray(np.asarray(q, dtype=np.float32))
    k = np.ascontiguousarray(np.asarray(k, dtype=np.float32))
    v = np.ascontiguousarray(np.asarray(v, dtype=np.float32))

    BF = ml_dtypes.bfloat16
    cos, sinr = _rope_tables()
    ck = np.ascontiguousarray(
        cos.reshape(T // 128, 128, HD).transpose(1, 0, 2)).astype(BF)
    sk = np.ascontiguousarray(
        sinr.reshape(T // 128, 128, HD).transpose(1, 0, 2)).astype(BF)

    in_maps = []
    for c in range(NCORES):
        b, qq = c // 4, c % 4
        qg = np.empty((128, NH * NQT_SLOT, HD), np.float32)
        kg = np.empty((128, NKT, HD), np.float32)
        vg = np.empty((128, NKT, HD + 1), np.float32)
        for h in range(NH):
            w, ko = WT[h], KOFF[h]
            a0 = T // 128 - w
            qs = q[b, qq * 512:(qq + 1) * 512, h * HD:(h + 1) * HD]
            qg[:, NQT_SLOT * h:NQT_SLOT * (h + 1), :] = (
                qs.reshape(NQT_SLOT, 128, HD).transpose(1, 0, 2))
            ks = k[b, a0 * 128:T, h * HD:(h + 1) * HD]
            kg[:, ko:ko + w, :] = ks.reshape(w, 128, HD).transpose(1, 0, 2)
            vs = v[b, a0 * 128:T, h * HD:(h + 1) * HD]
            eb = np.exp(SLOPES[h] * (np.arange(a0 * 128, T, dtype=np.float64)
                                     - (T - 1.0))).astype(np.float32)
            vsc = (vs * eb[:, None]).reshape(w, 128, HD).transpose(1, 0, 2)
            vg[:, ko:ko + w, 0:HD] = vsc
            vg[:, ko:ko + w, HD] = eb.reshape(w, 128).T
        cqg = np.ascontiguousarray(
            cos[qq * 512:(qq + 1) * 512].reshape(NQT_SLOT, 128, HD)
            .transpose(1, 0, 2)).astype(BF)
        sqg = np.ascontiguousarray(
            sinr[qq * 512:(qq + 1) * 512].reshape(NQT_SLOT, 128, HD)
            .transpose(1, 0, 2)).astype(BF)
        in_maps.append({
            "q_g": qg.astype(BF), "k_g": kg.astype(BF), "v_g": vg.astype(BF),
            "cos_q": cqg, "sin_q": sqg, "cos_k": ck, "sin_k": sk,
        })

    if _PROGRAM is None:
        _PROGRAM = _build_program()

    global LAST_RESULT
    res = run_bass_kernel_spmd(_PROGRAM, in_maps, core_ids=list(range(NCORES)),
                               trace=TRACE)
    LAST_RESULT = res

    out = np.empty((B, T, H), np.float32)
    for c in range(NCORES):
        b, qq = c // 4, c % 4
        og = res.results[c]["out_g"]
        for hi, h in enumerate(HEAD_ORDER):
            sl = og[:, NQT_SLOT * hi:NQT_SLOT * (hi + 1), :]   # [128, 4, 64]
            out[b, qq * 512:(qq + 1) * 512, h * HD:(h + 1) * HD] = (
                sl.transpose(1, 0, 2).reshape(512, HD))
    return out
